# revision 1
# baseline (speedup 1.0000x reference)
"""Fisher-Kolmogorov explicit-Euler solver (nn_DifferentiableEulerSolver) on 8
trn2 NeuronCores via Bass/Tile.

Strategy:
- Spatial decomposition: partitions = D (128), H sharded 8 x 16 rows per core,
  W contiguous (+1 zero pad col each side for the W-direction stencil shifts).
- Per micro-step per batch item:
    PSUM  = T0@u (d+-1 neighbor sum) + I@u(h-1) + I@u(h+1)   (PE, fp32 exact)
    SQ    = u^2                                              (ScalarE)
    W1    = u(w-1) + u(w+1); S = W1 + PSUM; CL = C*S         (DVE)
    AU = A*u; BS = Bt*SQ; T1 = AU+BS                         (GPSIMD)
    u'    = T1 + CL                                          (DVE)
  where A = 1 - 6*dt*D + dt*rho, Bt = -dt*rho, C = dt*D folded on host
  (the -6u Laplacian diagonal is absorbed into A).
- delta_t_days is read on the host: item b integrates delta_t_days[b]*10
  steps (masked steps in the reference are exact no-ops).
- Halo exchange per step: boundary rows (masked to zero at the global H
  edges) -> AllGather over all 8 cores -> per-core one-hot coefficient
  chains select the left/right neighbor slots (pure SPMD, no per-core
  control flow).
"""
import json as _json
import numpy as np
from contextlib import ExitStack

import bass_rust
from concourse import bass, tile
import concourse.mybir as mybir
from concourse.vector_clock import ScopedClock
from concourse.bass_utils import run_bass_kernel_spmd

N_CORES = 8
P = 128
HS = 16
R = HS + 2
W = 128
W2 = W + 2
DT = np.float32(0.1)
SUBSTEPS = 10

F32 = mybir.dt.float32
ALU = mybir.AluOpType
ACTF = mybir.ActivationFunctionType

# ---------------------------------------------------------------------------
# Workarounds for this neuronxcc: at most 1 semaphore wait per instruction.
# 1) TileContext's final drain carries one wait per ticked proc -> split onto
#    NoOps. 2) A JSON post-pass splits any remaining multi-wait instruction.
# ---------------------------------------------------------------------------
_PATCHED = False


def _patched_drain_and_barrier(self, tick_clock, wait_clock):
    nop = self.nc.sync.nop(nofuse=True, hint="split_drain_waits")
    wait_clock.add_sem_waits(nop.ins, ScopedClock({None: tick_clock.global_clock}))
    waits = list(nop.ins.sync_info.on_wait)
    if len(waits) > 1:
        nop.ins.sync_info = bass_rust.SyncInfo(
            on_wait=waits[:1], on_update=list(nop.ins.sync_info.on_update))
        for w in waits[1:]:
            n2 = self.nc.sync.nop(nofuse=True, hint="split_drain_waits")
            n2.ins.sync_info = bass_rust.SyncInfo(on_wait=[w], on_update=[])
    self.nc.sync.drain()
    self.nc.all_engine_barrier()
    assert self.sems is not None
    popped = self.nc._tile_sem_poison_stack.pop()
    assert popped is self._sem_poison
    self.nc.clear_and_free_semaphores(list(self.sems.allocated().values()))
    self.nc.all_engine_barrier()


def _split_waits_json(bir):
    ctr = [0]
    for fn in bir.get('functions', []):
        for blk in fn.get('blocks', []):
            out = []
            for inst in blk.get('instructions', []):
                si = inst.get('sync_info')
                waits = si.get('on_wait') if si else None
                if waits and len(waits) > 1:
                    for w in waits[:-1]:
                        ctr[0] += 1
                        out.append({
                            'debug': inst.get('debug'),
                            'engine': inst.get('engine'),
                            'ins': [], 'outs': [],
                            'name': f"wsplit{ctr[0]}_{inst['name']}",
                            'opcode': 'NoOp',
                            'sync_info': {'on_update': [], 'on_wait': [w]},
                        })
                    si['on_wait'] = waits[-1:]
                out.append(inst)
            blk['instructions'] = out
    return bir


def _install_patches():
    global _PATCHED
    if _PATCHED:
        return
    tile.TileContext._drain_and_barrier = _patched_drain_and_barrier
    orig = bass.Bass.to_json_bytes

    def patched_to_json_bytes(self, *a, **kw):
        bir = _json.loads(orig(self, *a, **kw))
        return _json.dumps(_split_waits_json(bir)).encode()

    bass.Bass.to_json_bytes = patched_to_json_bytes
    _PATCHED = True


# ---------------------------------------------------------------------------
# Program builder
# ---------------------------------------------------------------------------
_PROGRAM_CACHE = {}


def build_program(n_steps_per_item):
    key = tuple(n_steps_per_item)
    if key in _PROGRAM_CACHE:
        return _PROGRAM_CACHE[key]
    n_max = max(n_steps_per_item)
    assert n_max >= 1
    nc = bass.Bass(num_devices=N_CORES)

    u_in = nc.dram_tensor("u_in", [2, P, R, W2], F32, kind="ExternalInput")
    a_in = nc.dram_tensor("a_in", [2, P, HS, W], F32, kind="ExternalInput")
    b_in = nc.dram_tensor("b_in", [2, P, HS, W], F32, kind="ExternalInput")
    c_in = nc.dram_tensor("c_in", [2, P, HS, W], F32, kind="ExternalInput")
    wgt_in = nc.dram_tensor("wgt_in", [P, 2 * P], F32, kind="ExternalInput")
    mask_in = nc.dram_tensor("mask_in", [P, 2], F32, kind="ExternalInput")
    coef_in = nc.dram_tensor("coef_in", [P, 16], F32, kind="ExternalInput")
    y_out = nc.dram_tensor("y_out", [2, P, HS, W], F32, kind="ExternalOutput")

    cc_ins = [nc.dram_tensor(f"cc_in{par}", [P, 4, W2], F32) for par in range(2)]
    cc_outs = [nc.dram_tensor(f"cc_out{par}", [N_CORES, P, 4, W2], F32,
                              addr_space="Shared") for par in range(2)]

    with tile.TileContext(nc) as tc, ExitStack() as ctx:
        const = ctx.enter_context(tc.tile_pool(name="const", bufs=1))
        upool = ctx.enter_context(tc.tile_pool(name="upool", bufs=1))
        scratch = ctx.enter_context(tc.tile_pool(name="scratch", bufs=6))
        psum = ctx.enter_context(tc.tile_pool(name="psum", bufs=1, space="PSUM"))

        U = [[upool.tile([P, R, W2], F32, tag=f"u{par}_{b}", name=f"u{par}_{b}")
              for b in range(2)] for par in range(2)]
        A = [const.tile([P, HS, W], F32, tag=f"a{b}", name=f"a{b}")
             for b in range(2)]
        Bt = [const.tile([P, HS, W], F32, tag=f"b{b}", name=f"bt{b}")
              for b in range(2)]
        C = [const.tile([P, HS, W], F32, tag=f"c{b}", name=f"c{b}")
             for b in range(2)]
        WT = const.tile([P, 2 * P], F32, tag="wt")
        MSK = const.tile([P, 2], F32, tag="msk")
        COEF = const.tile([P, 16], F32, tag="coef", name="coef")
        stage = [const.tile([P, 4, W2], F32, tag=f"stage{par}",
                            name=f"stage{par}") for par in range(2)]

        for b in range(2):
            nc.sync.dma_start(out=U[0][b][:, :, :], in_=u_in[b])
            nc.sync.dma_start(out=A[b][:, :, :], in_=a_in[b])
            nc.sync.dma_start(out=Bt[b][:, :, :], in_=b_in[b])
            nc.sync.dma_start(out=C[b][:, :, :], in_=c_in[b])
        nc.sync.dma_start(out=WT[:, :], in_=wgt_in[:, :])
        nc.sync.dma_start(out=MSK[:, :], in_=mask_in[:, :])
        nc.sync.dma_start(out=COEF[:, :], in_=coef_in[:, :])
        for b in range(2):
            nc.vector.memset(U[1][b][:, :, :], 0.0)

        T0w = WT[:, 0:P]
        Iw = WT[:, P:2 * P]

        def interior(par, b, dr=0, dc=0):
            return U[par][b][:, 1 + dr:1 + dr + HS, 1 + dc:1 + dc + W]

        for s in range(n_max):
            p, q = s % 2, (s + 1) % 2
            active = [b for b in range(2) if s < n_steps_per_item[b]]
            for b in active:
                ps_q = [psum.tile([P, 4, W], F32, tag=f"ps{b}q{qi}", bufs=1,
                                  name=f"ps{b}q{qi}_{s}") for qi in range(4)]
                sq = scratch.tile([P, HS, W], F32, tag=f"scr{b}", name=f"sq{b}_{s}")
                w1 = scratch.tile([P, HS, W], F32, tag=f"scr{b}", name=f"w1{b}_{s}")
                ssum = scratch.tile([P, HS, W], F32, tag=f"scr{b}", name=f"ss{b}_{s}")
                cl = scratch.tile([P, HS, W], F32, tag=f"scr{b}", name=f"cl{b}_{s}")
                au = scratch.tile([P, HS, W], F32, tag=f"scr{b}", name=f"au{b}_{s}")
                bs = scratch.tile([P, HS, W], F32, tag=f"scr{b}", name=f"bs{b}_{s}")
                t1 = scratch.tile([P, HS, W], F32, tag=f"scr{b}", name=f"t1{b}_{s}")

                for ch in range(4):
                    r0 = 1 + 4 * ch
                    po = ps_q[ch][:, :, :]
                    nc.tensor.matmul(po, T0w, U[p][b][:, r0:r0 + 4, 1:1 + W],
                                     start=True, stop=False)
                    nc.tensor.matmul(po, Iw, U[p][b][:, r0 - 1:r0 + 3, 1:1 + W],
                                     start=False, stop=False)
                    nc.tensor.matmul(po, Iw, U[p][b][:, r0 + 1:r0 + 5, 1:1 + W],
                                     start=False, stop=True)

                nc.scalar.activation(sq[:, :, :], interior(p, b), ACTF.Square)
                nc.vector.tensor_tensor(
                    w1[:, :, :], interior(p, b, dc=-1), interior(p, b, dc=+1),
                    ALU.add)
                for qi in range(4):
                    nc.vector.tensor_tensor(
                        ssum[:, 4 * qi:4 * qi + 4, :],
                        w1[:, 4 * qi:4 * qi + 4, :], ps_q[qi][:, :, :], ALU.add)
                nc.vector.tensor_tensor(
                    cl[:, :, :], C[b][:, :, :], ssum[:, :, :], ALU.mult)
                nc.gpsimd.tensor_tensor(
                    au[:, :, :], A[b][:, :, :], interior(p, b), ALU.mult)
                nc.gpsimd.tensor_tensor(
                    bs[:, :, :], Bt[b][:, :, :], sq[:, :, :], ALU.mult)
                nc.gpsimd.tensor_tensor(
                    t1[:, :, :], au[:, :, :], bs[:, :, :], ALU.add)
                nc.vector.tensor_tensor(
                    interior(q, b), t1[:, :, :], cl[:, :, :], ALU.add)

            if s < n_max - 1:
                par = s % 2
                st = stage[par]
                for b in active:
                    nc.vector.tensor_scalar(
                        st[:, 2 * b + 0, :], U[q][b][:, 1, :],
                        MSK[:, 0:1], None, ALU.mult)
                    nc.vector.tensor_scalar(
                        st[:, 2 * b + 1, :], U[q][b][:, HS, :],
                        MSK[:, 1:2], None, ALU.mult)
                nc.sync.dma_start(out=cc_ins[par][:, :, :], in_=st[:, :, :])
                nc.gpsimd.collective_compute(
                    "AllGather", ALU.bypass,
                    replica_groups=[list(range(N_CORES))],
                    ins=[cc_ins[par][:, :, :]],
                    outs=[cc_outs[par][:, :, :, :]],
                )
                rcv = scratch.tile([P, N_CORES, 4, W2], F32, tag="rcv",
                                   name=f"rcv_{s}", bufs=1)
                for sl in range(N_CORES):
                    nc.sync.dma_start(out=rcv[:, sl, :, :], in_=cc_outs[par][sl])
                for b in active:
                    for side, row in ((1, 0), (0, R - 1)):
                        co = 0 if row == 0 else 8
                        j = 2 * b + side
                        hprev = None
                        for sl in range(N_CORES):
                            last = sl == N_CORES - 1
                            dst = (U[q][b][:, row, :] if last else
                                   scratch.tile([P, W2], F32, tag="hrow",
                                                name=f"h_{s}_{b}_{row}_{sl}",
                                                bufs=4))
                            if hprev is None:
                                nc.vector.tensor_scalar(
                                    dst if last else dst[:, :],
                                    rcv[:, sl, j, :],
                                    COEF[:, co + sl:co + sl + 1],
                                    None, ALU.mult)
                            else:
                                nc.vector.scalar_tensor_tensor(
                                    dst if last else dst[:, :],
                                    rcv[:, sl, j, :],
                                    COEF[:, co + sl:co + sl + 1],
                                    hprev, ALU.mult, ALU.add)
                            hprev = None if last else dst[:, :]

        for b in range(2):
            fin = n_steps_per_item[b] % 2
            out_t = scratch.tile([P, HS, W], F32, tag=f"scr{b}", name=f"fin{b}")
            nc.vector.tensor_scalar(
                out_t[:, :, :], interior(fin, b), 0.0, 1.0, ALU.max, ALU.min)
            nc.sync.dma_start(out=y_out[b], in_=out_t[:, :, :])

    _PROGRAM_CACHE[key] = nc
    return nc


def _coef_for_core(i):
    c = np.zeros(16, np.float32)
    c[(i - 1) % 8] = 1.0
    c[8 + (i + 1) % 8] = 1.0
    return np.broadcast_to(c, (P, 16)).copy()


def make_inputs(u_t0, D_map, rho_map):
    u = u_t0[:, 0].astype(np.float32)
    Dm = D_map[:, 0].astype(np.float32)
    Rm = rho_map[:, 0].astype(np.float32)
    Cf = (DT * Dm).astype(np.float32)
    Bf = (-(DT * Rm)).astype(np.float32)
    Af = (np.float32(1.0) - np.float32(6.0) * DT * Dm + DT * Rm).astype(np.float32)

    T0 = np.zeros((P, P), np.float32)
    for k in range(P - 1):
        T0[k, k + 1] = 1.0
        T0[k + 1, k] = 1.0
    wgt = np.concatenate([T0, np.eye(P, dtype=np.float32)], axis=1)

    ins = []
    for i in range(N_CORES):
        h0 = HS * i
        up = np.zeros((2, P, R, W2), np.float32)
        up[:, :, 1:1 + HS, 1:1 + W] = u[:, :, h0:h0 + HS, :]
        if i > 0:
            up[:, :, 0, 1:1 + W] = u[:, :, h0 - 1, :]
        if i < N_CORES - 1:
            up[:, :, R - 1, 1:1 + W] = u[:, :, h0 + HS, :]
        sl = np.s_[:, :, h0:h0 + HS, :]
        ins.append({
            "u_in": up,
            "a_in": np.ascontiguousarray(Af[sl]),
            "b_in": np.ascontiguousarray(Bf[sl]),
            "c_in": np.ascontiguousarray(Cf[sl]),
            "wgt_in": wgt,
            "mask_in": np.stack([
                np.full(P, 0.0 if i == 0 else 1.0, np.float32),
                np.full(P, 0.0 if i == N_CORES - 1 else 1.0, np.float32),
            ], axis=1),
            "coef_in": _coef_for_core(i),
        })
    return ins


def kernel(u_t0, D_map, rho_map, delta_t_days):
    u_t0 = np.asarray(u_t0, dtype=np.float32)
    D_map = np.asarray(D_map, dtype=np.float32)
    rho_map = np.asarray(rho_map, dtype=np.float32)
    delta_t_days = np.asarray(delta_t_days)
    nsi = [int(delta_t_days[b]) * SUBSTEPS for b in range(2)]

    if max(nsi) == 0:
        return np.clip(u_t0, 0.0, 1.0).astype(np.float32)

    _install_patches()
    nc = build_program(nsi)
    ins = make_inputs(u_t0, D_map, rho_map)
    res = run_bass_kernel_spmd(nc, ins, list(range(N_CORES)))

    out = np.zeros((2, 1, 128, 128, 128), np.float32)
    for i in range(N_CORES):
        out[:, 0, :, HS * i:HS * (i + 1), :] = res.results[i]["y_out"]
    return out



# revision 13
# speedup vs baseline: 1.4374x; 1.4374x over previous
"""Fisher-Kolmogorov explicit-Euler solver (nn_DifferentiableEulerSolver) on 8
trn2 NeuronCores via Bass/Tile.

Strategy (zero-communication halo decay):
- Shard B x H: core i = 4*b + k handles item b, owned H rows [32k, 32k+32).
  Each core loads its owned slab plus a 30-row halo on each side (clamped by
  the global Dirichlet-zero boundary) and runs all micro-steps locally with a
  shrinking compute window -- after 30 steps the owned 32 rows are exact with
  NO inter-core communication (the previous AllGather-per-step design spent
  ~20 ms per step in the collective).
- Layout: partitions = D (128), free dims = (H rows, W cols), W padded by a
  zero column each side; one zero pad row above/below the 92-row slab.
- Per micro-step (ping-pong u buffers):
    PSUM[c] = T0'@u + I@u(h-1) + I@u(h+1) + I@u(w-1) + I@u(w+1)   (PE, fp32r)
              -- T0' has -6 on the diagonal, so PSUM = 6-neighbor sum - 6u
    SQ   = Square(act*u - 0.5*act)          (Act)     = act*(u-0.5)^2
    SQ   = (SQ - 0.25*act) * PM             (Pool)    = -act*dt*rho*u*(1-u)
    uq[c]= (PSUM[c] * act) * CM             (DVE)     = act*dt*D*lap(u)
    uq   = uq - SQ                          (DVE)
    uq   = uq + u                           (Pool)
  where CM = dt*D_map, PM = dt*rho_map (bf16 maps), act in {0,1} masks
  inactive trailing steps per item (exact no-ops, matching the reference).
- Compute windows shrink triangularly: step s only needs rows
  [32-n+s, 61+n-s] (4-row-chunk aligned), which also keeps the tail cheap.
"""
import json as _json
import numpy as np
from contextlib import ExitStack

import ml_dtypes
import bass_rust
from concourse import bass, tile
import concourse.mybir as mybir
from concourse.vector_clock import ScopedClock
from concourse.bass_utils import run_bass_kernel_spmd

N_CORES = 8
P = 128            # partitions = D
KH = 4             # H-split per item
OWN = 32           # owned H rows per core
HALO = 30          # max steps = (MAX_DAYS-1)*SUBSTEPS
RI = OWN + 2 * HALO   # 92 interior rows
RB = RI + 2           # + zero pad row each side
W = 128
W2 = W + 2
DT = np.float32(0.1)
SUBSTEPS = 10

F32 = mybir.dt.float32
F32R = mybir.dt.float32r
BF16 = mybir.dt.bfloat16
ALU = mybir.AluOpType
ACTF = mybir.ActivationFunctionType

# ---------------------------------------------------------------------------
# Workarounds for this neuronxcc: at most 1 semaphore wait per instruction.
# 1) TileContext's final drain carries one wait per ticked proc -> split onto
#    NoOps. 2) A JSON post-pass splits any remaining multi-wait instruction.
# ---------------------------------------------------------------------------
_PATCHED = False


def _patched_drain_and_barrier(self, tick_clock, wait_clock):
    nop = self.nc.sync.nop(nofuse=True, hint="split_drain_waits")
    wait_clock.add_sem_waits(nop.ins, ScopedClock({None: tick_clock.global_clock}))
    waits = list(nop.ins.sync_info.on_wait)
    if len(waits) > 1:
        nop.ins.sync_info = bass_rust.SyncInfo(
            on_wait=waits[:1], on_update=list(nop.ins.sync_info.on_update))
        for w in waits[1:]:
            n2 = self.nc.sync.nop(nofuse=True, hint="split_drain_waits")
            n2.ins.sync_info = bass_rust.SyncInfo(on_wait=[w], on_update=[])
    self.nc.sync.drain()
    self.nc.all_engine_barrier()
    assert self.sems is not None
    popped = self.nc._tile_sem_poison_stack.pop()
    assert popped is self._sem_poison
    self.nc.clear_and_free_semaphores(list(self.sems.allocated().values()))
    self.nc.all_engine_barrier()


def _split_waits_json(bir):
    ctr = [0]
    for fn in bir.get('functions', []):
        for blk in fn.get('blocks', []):
            out = []
            for inst in blk.get('instructions', []):
                si = inst.get('sync_info')
                waits = si.get('on_wait') if si else None
                if waits and len(waits) > 1:
                    for w in waits[:-1]:
                        ctr[0] += 1
                        out.append({
                            'debug': inst.get('debug'),
                            'engine': inst.get('engine'),
                            'ins': [], 'outs': [],
                            'name': f"wsplit{ctr[0]}_{inst['name']}",
                            'opcode': 'NoOp',
                            'sync_info': {'on_update': [], 'on_wait': [w]},
                        })
                    si['on_wait'] = waits[-1:]
                out.append(inst)
            blk['instructions'] = out
    return bir


def _install_patches():
    global _PATCHED
    if _PATCHED:
        return
    tile.TileContext._drain_and_barrier = _patched_drain_and_barrier
    orig = bass.Bass.to_json_bytes

    def patched_to_json_bytes(self, *a, **kw):
        bir = _json.loads(orig(self, *a, **kw))
        return _json.dumps(_split_waits_json(bir)).encode()

    bass.Bass.to_json_bytes = patched_to_json_bytes
    _PATCHED = True


# ---------------------------------------------------------------------------
# Program builder (one SPMD program; per-core behavior comes from the inputs)
# ---------------------------------------------------------------------------
_PROGRAM_CACHE = {}


def _windows(n_max):
    """Per-step (c_lo, c_hi, r0, r1): 4-row chunk range and the covered
    (inclusive) buffer-row range for step s of an n_max-step rollout."""
    out = []
    for s in range(n_max):
        lo = 32 - n_max + s
        hi = 61 + n_max - s
        c_lo = max(0, (lo - 1) // 4)
        c_hi = min((RI // 4) - 1, (hi - 1) // 4)
        out.append((c_lo, c_hi, 1 + 4 * c_lo, 4 + 4 * c_hi))
    return out


def build_program(n_max):
    if n_max in _PROGRAM_CACHE:
        return _PROGRAM_CACHE[n_max]
    assert 1 <= n_max <= HALO

    nc = bass.Bass(num_devices=N_CORES)

    u_in = nc.dram_tensor("u_in", [P, RB, W2], F32R, kind="ExternalInput")
    cm_in = nc.dram_tensor("cm_in", [P, RI, W], BF16, kind="ExternalInput")
    pm_in = nc.dram_tensor("pm_in", [P, RI, W], BF16, kind="ExternalInput")
    wgt_in = nc.dram_tensor("wgt_in", [P, 2 * P], F32R, kind="ExternalInput")
    act_in = nc.dram_tensor("act_in", [P, 3 * n_max], F32, kind="ExternalInput")
    y_out = nc.dram_tensor("y_out", [P, OWN, W], F32R, kind="ExternalOutput")

    with tile.TileContext(nc) as tc, ExitStack() as ctx:
        const = ctx.enter_context(tc.tile_pool(name="const", bufs=1))
        upool = ctx.enter_context(tc.tile_pool(name="upool", bufs=1))
        scratch = ctx.enter_context(tc.tile_pool(name="scratch", bufs=1))
        psum = ctx.enter_context(tc.tile_pool(name="psum", bufs=1, space="PSUM"))

        U = [upool.tile([P, RB, W2], F32R, tag=f"u{i}", name=f"u{i}")
             for i in range(2)]
        CM = const.tile([P, RI, W], BF16, tag="cm", name="cm")
        PM = const.tile([P, RI, W], BF16, tag="pm", name="pm")
        WT = const.tile([P, 2 * P], F32R, tag="wt", name="wt")
        ACT = const.tile([P, 3 * n_max], F32, tag="act", name="act")
        SQ = scratch.tile([P, RI, W], BF16, tag="sq", name="sq")
        T1 = scratch.tile([P, RI, W], BF16, tag="t1", name="t1")

        nc.sync.dma_start(out=U[0][:, :, :], in_=u_in[:, :, :])
        nc.sync.dma_start(out=CM[:, :, :], in_=cm_in[:, :, :])
        nc.sync.dma_start(out=PM[:, :, :], in_=pm_in[:, :, :])
        nc.sync.dma_start(out=WT[:, :], in_=wgt_in[:, :])
        nc.sync.dma_start(out=ACT[:, :], in_=act_in[:, :])
        nc.vector.tensor_scalar(U[1][:, :, :], U[0][:, :, :], 0.0, None, ALU.mult)

        T0r = WT[:, 0:P]
        Ir = WT[:, P:2 * P]

        NP4 = RI // 4  # 23 chunks of 4 rows

        for s, (c_lo, c_hi, r0, r1) in enumerate(_windows(n_max)):
            p, q = s % 2, (s + 1) % 2
            Up, Uq = U[p], U[q]
            a_s = ACT[:, s:s + 1]
            b_s = ACT[:, n_max + s:n_max + s + 1]       # -0.5*act
            c_s = ACT[:, 2 * n_max + s:2 * n_max + s + 1]  # 0.25*act

            # 16-row chunk quads (4 PSUM banks per tile); quad j = chunks
            # (4j..4j+3); quad 5 holds only chunks 20-22.
            pj_lo, pj_hi = c_lo // 4, c_hi // 4
            pairs = list(range(pj_lo, pj_hi + 1))
            # one quad per block
            blocks = [pairs[i:i + 1] for i in range(0, len(pairs), 1)]
            # snake: alternate sweep direction per step so the next step's
            # first chunks depend on this step's freshest rows
            if s % 2 == 1:
                blocks = [list(reversed(b)) for b in reversed(blocks)]

            for bi, blk in enumerate(blocks):
                ja, jb = min(blk), max(blk)
                b0, b1 = 16 * ja + 1, min(16 * jb + 16, RI)  # buffer rows
                # reaction: SQ = act*(u-.5)^2; (SQ-.25a); *PM
                nc.scalar.activation(SQ[:, b0 - 1:b1, :],
                                     Up[:, b0:b1 + 1, 1:1 + W],
                                     ACTF.Square, bias=b_s, scale=a_s)
                nc.vector.tensor_scalar(SQ[:, b0 - 1:b1, :], SQ[:, b0 - 1:b1, :],
                                        c_s, None, ALU.subtract)
                nc.vector.tensor_tensor(SQ[:, b0 - 1:b1, :], SQ[:, b0 - 1:b1, :],
                                        PM[:, b0 - 1:b1, :], ALU.mult)
                # diffusion via PE: psum = 6-neighbor sum - 6u per 4-row chunk
                for j in blk:
                    ps = psum.tile([P, 16, W], F32, tag=f"ps{j % 2}", bufs=1,
                                   name=f"ps{s}_{j}")
                    chunks = [c for c in range(4 * j, 4 * j + 4) if c < NP4]
                    for c in chunks:
                        rc = 1 + 4 * c
                        o = 4 * (c - 4 * j)
                        po = ps[:, o:o + 4, :]
                        nc.tensor.matmul(po, T0r, Up[:, rc:rc + 4, 1:1 + W],
                                         start=True, stop=False)
                        nc.tensor.matmul(po, Ir, Up[:, rc - 1:rc + 3, 1:1 + W],
                                         start=False, stop=False)
                        nc.tensor.matmul(po, Ir, Up[:, rc + 1:rc + 5, 1:1 + W],
                                         start=False, stop=False)
                        nc.tensor.matmul(po, Ir, Up[:, rc:rc + 4, 0:W],
                                         start=False, stop=False)
                        nc.tensor.matmul(po, Ir, Up[:, rc:rc + 4, 2:2 + W],
                                         start=False, stop=True)
                    nr = 4 * len(chunks)
                    pr = 16 * j  # T1/map row of quad start
                    # Act drains PSUM (applying act) so DVE stays free
                    nc.scalar.activation(T1[:, pr:pr + nr, :], ps[:, 0:nr, :],
                                         ACTF.Copy, bias=0.0, scale=a_s)
                # T1 *= CM (bf16 2x); DL = T1 - SQ (bf16 2x); uq = DL + u
                nc.vector.tensor_tensor(
                    T1[:, b0 - 1:b1, :], T1[:, b0 - 1:b1, :],
                    CM[:, b0 - 1:b1, :], ALU.mult)
                nc.vector.tensor_tensor(
                    T1[:, b0 - 1:b1, :], T1[:, b0 - 1:b1, :],
                    SQ[:, b0 - 1:b1, :], ALU.subtract)
                # the last block feeds the next step first: put it on DVE
                eng = nc.vector if bi == len(blocks) - 1 else nc.gpsimd
                eng.tensor_tensor(
                    Uq[:, b0:b1 + 1, 1:1 + W], T1[:, b0 - 1:b1, :],
                    Up[:, b0:b1 + 1, 1:1 + W], ALU.add)

        fin = U[n_max % 2]
        dead = U[(n_max + 1) % 2]
        nc.vector.tensor_scalar(dead[:, 31:31 + OWN, 1:1 + W],
                                fin[:, 31:31 + OWN, 1:1 + W],
                                0.0, 1.0, ALU.max, ALU.min)
        nc.sync.dma_start(out=y_out[:, :, :], in_=dead[:, 31:31 + OWN, 1:1 + W])

    _PROGRAM_CACHE[n_max] = nc
    return nc


# ---------------------------------------------------------------------------
# Host-side sharding
# ---------------------------------------------------------------------------

def make_inputs(u_t0, D_map, rho_map, nsi, n_max):
    u = u_t0[:, 0].astype(np.float32)
    cm_full = (DT * D_map[:, 0]).astype(ml_dtypes.bfloat16)
    pm_full = (DT * rho_map[:, 0]).astype(ml_dtypes.bfloat16)

    T0p = np.zeros((P, P), np.float32)
    for k in range(P - 1):
        T0p[k, k + 1] = 1.0
        T0p[k + 1, k] = 1.0
    np.fill_diagonal(T0p, -6.0)
    wgt = np.concatenate([T0p, np.eye(P, dtype=np.float32)], axis=1)

    ins = []
    for i in range(N_CORES):
        b, k = divmod(i, KH)
        g0 = OWN * k - HALO
        rlo, rhi = max(g0, 0), min(g0 + RI, P)

        up = np.zeros((P, RB, W2), np.float32)
        up[:, rlo - g0 + 1:rhi - g0 + 1, 1:1 + W] = u[b][:, rlo:rhi, :]
        cm = np.zeros((P, RI, W), ml_dtypes.bfloat16)
        cm[:, rlo - g0:rhi - g0, :] = cm_full[b][:, rlo:rhi, :]
        pm = np.zeros((P, RI, W), ml_dtypes.bfloat16)
        pm[:, rlo - g0:rhi - g0, :] = pm_full[b][:, rlo:rhi, :]

        a = (np.arange(n_max) < nsi[b]).astype(np.float32)
        act = np.concatenate([a, -0.5 * a, 0.25 * a])
        ins.append({
            "u_in": up,
            "cm_in": cm,
            "pm_in": pm,
            "wgt_in": wgt,
            "act_in": np.broadcast_to(act, (P, 3 * n_max)).copy(),
        })
    return ins


def kernel(u_t0, D_map, rho_map, delta_t_days):
    u_t0 = np.asarray(u_t0, dtype=np.float32)
    D_map = np.asarray(D_map, dtype=np.float32)
    rho_map = np.asarray(rho_map, dtype=np.float32)
    delta_t_days = np.asarray(delta_t_days)
    nsi = [int(delta_t_days[b]) * SUBSTEPS for b in range(2)]
    n_max = max(nsi)

    if n_max == 0:
        return np.clip(u_t0, 0.0, 1.0).astype(np.float32)

    _install_patches()
    nc = build_program(n_max)
    ins = make_inputs(u_t0, D_map, rho_map, nsi, n_max)
    res = run_bass_kernel_spmd(nc, ins, list(range(N_CORES)))

    out = np.zeros((2, 1, P, P, W), np.float32)
    for i in range(N_CORES):
        b, k = divmod(i, KH)
        out[b, 0, :, OWN * k:OWN * (k + 1), :] = res.results[i]["y_out"]
    return out
